# revision 10
# baseline (speedup 1.0000x reference)
"""DOTA mix E-step (vq_codebook) on 8 TRN2 NeuronCores.

out[b,k,m] = gamma_class[b,k] * softmax_m(-0.5*(log_det+maha) + log_pi)

Implicit-reference formulation: softmax over modes is shift-invariant, so
each class pins one reference mode r (the one with the largest constant
term) at logit 0 and the GEMM computes only the c-1 DIFFERENCE logits

  l''[b,j] = x2 . (W1_m - W1_r + dlc) + x . (W2_m - W2_r)

with W1 = -0.5/var, W2 = mu/var and the per-column constant dlc folded
uniformly into W1diff (legal since sum_d x^2 = 1 for unit-norm x). Then
s = 1 + sum_j exp(l''), rec = 1/s, and the HOST computes
out_m = gamma*rec*e_m (m != r), out_r = gamma*rec during the scatter:
the device returns the raw exps in bf16 (range to e^88, so the f32
logit bound of 75 can never overflow) plus rec in f32. That removes the
whole device-side output-scaling stage and the gamma traffic.

Sharding is 2-way over batch x 4-way over classes: per-core DMA drops to
~9MB and per-chunk exp becomes one wide Activation instruction over both
PSUM banks. Classes are bucketed by width c-1, rounded to multiples of 4
by promoting classes from the next-lower pool (one wasted -20000 column
each) so all cores run one SPMD program. Count-1 classes are exact on
host (resp = 1).

Device pipeline per 128-row chunk: the x2@W1diff GEMM runs in fp8 e4m3
DoubleRow (x2 scaled by 64 into e4m3's normal range, W1diff/64), x@W2diff
in f16, split over two PSUM banks; exp on ScalarE straight from both
banks into a bf16 tile that is DMAed out unmodified. Per-class sums run
as one segmented reduce per width bucket (DVE mostly, widest on GPSIMD),
then +1 on ScalarE (Copy w/ bias) and fast reciprocal on DVE. A long
single-accumulation warmup keeps the PE's HAM clock-gate at 2.4GHz while
the startup DMAs land.
"""

import sys

import ml_dtypes
import numpy as np

sys.path.insert(0, "/opt/trn_rl_repo")

import concourse.bass as bass
import concourse.mybir as mybir
import concourse.tile as tile
from concourse import bacc, bass_utils

F32 = mybir.dt.float32
F16 = mybir.dt.float16
BF16 = mybir.dt.bfloat16
F8 = mybir.dt.float8e4

X2S = 8.0          # x2 scaled by X2S^2=64 into e4m3's normal range

B, K, M, D = 4096, 1000, 8, 512
NCORES = 8
BSH = 2                   # batch shards
KSH = 4                   # class shards
RB = B // BSH             # 2048 rows per core
NB = RB // 128            # 16 batch chunks of 128 rows
NQ = NB // 4              # x loaded in quads of 4 chunks
GROUPS = (2, 2, 4, 4, 2, 1, 1)  # small first groups start the DVE
                                # post-pipeline early; tapered tail
NWARM = 56
EPS_REG = 1e-3
PAD_LOGIT = -20000.0      # exp -> 0 for promoted/dummy columns
MAX_LOGIT = 75.0          # overflow guard for exp in f32
POOL_W = ()               # X-axis reduce is DVE-only on TRN2


def build_bass(buckets):
    """buckets: tuple of (width, n_classes_per_core) for widths 1..7."""
    nv = sum(w * n for w, n in buckets)       # packed diff columns per core
    kc = sum(n for _, n in buckets)           # packed classes per core
    assert nv <= 1024, nv
    GMAX = max(GROUPS)
    assert sum(GROUPS) == NB

    nvp = (nv + 15) // 16 * 16    # 16-elem-aligned i-stride for DoubleRow

    nc = bacc.Bacc("TRN2", debug=False, target_bir_lowering=False)
    # x stored (r, d, bc, j) so a 4-chunk load reads 1KB-contiguous segments
    xt = nc.dram_tensor("xt", (4, 128, NB, 128), F16, kind="ExternalInput")
    x2t = nc.dram_tensor("x2t", (4, 128, NB, 128), F8, kind="ExternalInput")
    w1 = nc.dram_tensor("w1", (2, 128, 2 * nvp), F8, kind="ExternalInput")
    w2 = nc.dram_tensor("w2", (4, 128, nv), F16, kind="ExternalInput")
    out = nc.dram_tensor("out", (RB, nv), BF16, kind="ExternalOutput")
    cf = nc.dram_tensor("cf", (RB, kc), F32, kind="ExternalOutput")
    warm = nc.dram_tensor("warm", (128, 128), F32, kind="ExternalOutput")

    xt_ap, x2t_ap, out_ap, cf_ap = (xt.ap(), x2t.ap(), out.ap(), cf.ap())

    nbank = (nv + 511) // 512
    bank_cols = [(i * 512, min(nv, (i + 1) * 512)) for i in range(nbank)]

    with tile.TileContext(nc) as tc:
        with (
            tc.tile_pool(name="wpool", bufs=1) as wpool,
            tc.tile_pool(name="xpool", bufs=3) as xpool,
            tc.tile_pool(name="ppool", bufs=3, space="PSUM") as ppool,
            tc.tile_pool(name="wppool", bufs=1, space="PSUM") as wppool,
            tc.tile_pool(name="epool", bufs=4) as epool,
            tc.tile_pool(name="spool", bufs=3) as spool,
        ):
            # warmup weights first so the HAM warmup can start immediately
            wz = wpool.tile([128, 128], F16, tag="warmz")
            nc.gpsimd.memset(wz[:], 0.0)

            # weight tiles: w1 on the scalar ring, w2 on the gpsimd SWDGE
            # ring — both parallel to the x loads on sync/vector
            w1t = []
            for r in range(2):
                t = wpool.tile([128, 2 * nvp], F8, tag=f"w1_{r}")
                nc.scalar.dma_start(t[:], w1.ap()[r])
                w1t.append(t)
            w2t = []
            for r in range(4):
                t = wpool.tile([128, nv], F16, tag=f"w2_{r}")
                nc.gpsimd.dma_start(t[:], w2.ap()[r])
                w2t.append(t)

            # x octo loads (8 chunks each, 2KB descriptors): x on the sync
            # ring, pre-squared fp8 x^2 on the vector ring — few triggers,
            # parallel queues, so the first chunk's data lands fast
            xtiles = {}

            def load_oct(p):
                if p >= NB // 8:
                    return
                xq = xpool.tile([128, 4096], F16, tag="xb")
                nc.sync.dma_start(
                    xq[:].rearrange("p (r c j) -> p r c j", r=4, c=8),
                    xt_ap[:, :, 8 * p:8 * p + 8].rearrange(
                        "r p c j -> p r c j"))
                x2q = xpool.tile([128, 4096], F8, tag="x2b")
                nc.gpsimd.dma_start(
                    x2q[:].rearrange("p (r c j) -> p r c j", r=4, c=8),
                    x2t_ap[:, :, 8 * p:8 * p + 8].rearrange(
                        "r p c j -> p r c j"))
                xtiles[p] = (xq, x2q)

            load_oct(0)
            load_oct(1)

            # HAM warmup: dummy matmuls while DMAs land, so the real GEMM
            # starts at 2.4 GHz instead of 1.2. One accumulation group so
            # consecutive MMs pipeline at N cycles instead of paying a full
            # fill+drain each. The warm result goes out on the scalar ring
            # so it never blocks the x quads queued on sync.
            wps = wppool.tile([128, 512], F32, tag="wps")
            for i in range(NWARM):
                nc.tensor.matmul(wps[:, 0:128], lhsT=wz[:], rhs=wz[:],
                                 start=(i == 0), stop=(i == NWARM - 1))
            wsb = wpool.tile([128, 128], F32, tag="warmsb")
            nc.vector.tensor_copy(wsb[:], wps[:, 0:128])
            nc.scalar.dma_start(warm.ap()[:, :], wsb[:])

            NG = len(GROUPS)
            gstart = [sum(GROUPS[:i]) for i in range(NG)]

            for g in range(NG):
                g0, Gk = gstart[g], GROUPS[g]
                ew = epool.tile([128, GMAX * nv], BF16, tag="ew")

                for q in range(Gk):
                    bc = g0 + q
                    if bc % 8 == 0:
                        xq, x2q = xtiles.pop(bc // 8)
                    cq = (bc % 8) * 128
                    ps = ppool.tile([128, 512 * nbank], F32, tag="ps")
                    x2v = x2q[:].rearrange("p (r c j) -> p r c j", r=4, c=8)
                    for c0, c1 in bank_cols:
                        for pr in range(2):
                            nc.tensor.matmul(
                                ps[:, c0:c1],
                                lhsT=x2v[:, 2 * pr:2 * pr + 2, bc % 8, :],
                                rhs=w1t[pr][:].rearrange(
                                    "p (i v) -> p i v", i=2)[:, :, c0:c1],
                                start=(pr == 0), stop=False,
                                perf_mode=mybir.MatmulPerfMode.DoubleRow)
                        for r in range(4):
                            nc.tensor.matmul(
                                ps[:, c0:c1],
                                lhsT=xq[:, r * 1024 + cq:r * 1024 + cq + 128],
                                rhs=w2t[r][:, c0:c1],
                                start=False, stop=(r == 3))
                    # one exp over both PSUM banks -> raw bf16 exps
                    nc.scalar.activation(ew[:, q * nv:(q + 1) * nv],
                                         ps[:, 0:nv],
                                         mybir.ActivationFunctionType.Exp)

                # per-class sums: one segmented reduce per width bucket
                ew3 = ew[:, 0:Gk * nv].rearrange("p (c v) -> p c v", v=nv)
                ssum = spool.tile([128, GMAX * kc], F32, tag="ssum")
                ss3 = ssum[:, 0:Gk * kc].rearrange("p (c k) -> p c k", c=Gk)
                off = koff = 0
                for w, n in buckets:
                    eng = nc.gpsimd if w in POOL_W else nc.vector
                    eng.reduce_sum(
                        ss3[:, :, koff:koff + n],
                        ew3[:, :, off:off + n * w].rearrange(
                            "p c (k m) -> p c k m", m=w),
                        axis=mybir.AxisListType.X)
                    off += n * w
                    koff += n
                # +1 on ScalarE (Copy w/ bias), fast reciprocal on DVE
                s1 = spool.tile([128, GMAX * kc], F32, tag="s1")
                nc.scalar.activation(s1[:, 0:Gk * kc], ssum[:, 0:Gk * kc],
                                     mybir.ActivationFunctionType.Copy,
                                     bias=1.0)
                rec = spool.tile([128, GMAX * kc], F32, tag="rec")
                nc.vector.reciprocal_approx_fast(
                    rec[:, 0:Gk * kc], s1[:, 0:Gk * kc])

                rows = slice(g0 * 128, (g0 + Gk) * 128)
                nc.scalar.dma_start(
                    out_ap[rows, :].rearrange("(c p) j -> p c j", p=128),
                    ew[:, 0:Gk * nv].rearrange("p (c j) -> p c j", c=Gk))
                nc.sync.dma_start(
                    cf_ap[rows, :].rearrange("(c p) k -> p c k", p=128),
                    rec[:, 0:Gk * kc].rearrange("p (c k) -> p c k", c=Gk))

    nc.compile()
    return nc


def _layout(mask):
    """Bucket classes by diff-width w = count-1 (count-1 classes are host
    handled); round each bucket to a multiple of KSH by promoting classes
    from the next-lower pool (cost: 1 wasted column each); remaining gaps in
    the w=1 bucket get dummies (-1)."""
    counts = np.asarray(mask, bool).sum(-1).astype(int)     # (K,)
    pools = {w: list(np.where(counts == w + 1)[0]) for w in range(1, M)}
    entries = []
    for w in range(M - 1, 0, -1):
        ids = pools[w]
        pools[w] = []
        pad = (-len(ids)) % KSH
        if pad and w > 1 and len(pools[w - 1]) >= pad:
            ids += pools[w - 1][:pad]
            pools[w - 1] = pools[w - 1][pad:]
        elif pad:
            ids += [-1] * pad
        if ids:
            entries.append((w, ids))
    entries.sort()
    per_shard = [[] for _ in range(KSH)]
    buckets = []
    for w, ids in entries:
        n = len(ids) // KSH
        buckets.append((w, n))
        for c in range(KSH):
            per_shard[c].append((w, ids[c * n:(c + 1) * n]))
    ones = np.where(counts == 1)[0]
    return tuple(buckets), per_shard, ones


def prep_inputs(x, gamma_class, mu_pad, var_pad, pi_pad, mask):
    x = np.asarray(x, np.float32)
    mask = np.asarray(mask, bool)
    counts = mask.sum(-1).astype(int)

    var = np.clip(np.asarray(var_pad, np.float64) + EPS_REG, 1e-8, None)
    inv = 1.0 / var
    W1 = -0.5 * inv                                    # (K, M, D)
    W2 = np.asarray(mu_pad, np.float64) * inv
    logdet = np.log(var).sum(-1)
    muinvmu = (np.asarray(mu_pad, np.float64) * W2).sum(-1)
    logpi = np.where(mask, np.log(np.asarray(pi_pad, np.float64) + 1e-10),
                     -np.inf)
    lc = -0.5 * logdet - 0.5 * muinvmu + logpi          # (K, M)

    lc_valid = np.where(mask, lc, -np.inf)
    ref = np.argmax(lc_valid, axis=1)                   # (K,)

    def class_bound(k, r):
        c = counts[k]
        ms = [m for m in range(c) if m != r]
        if not ms:
            return -np.inf
        dW1 = W1[k, ms] - W1[k, r] + (lc[k, ms] - lc[k, r])[:, None]
        dW2 = W2[k, ms] - W2[k, r]
        return (dW1.max(-1) + np.sqrt((dW2 ** 2).sum(-1))).max()

    # overflow guard: exp stays finite in f32; re-pick ref if needed
    for k in np.where(counts >= 2)[0]:
        if class_bound(k, ref[k]) > MAX_LOGIT:
            cand = [(class_bound(k, r), r) for r in range(counts[k])]
            bd, r = min(cand)
            if bd > MAX_LOGIT:
                raise ValueError(f"class {k}: logit bound {bd:.1f} > "
                                 f"{MAX_LOGIT}; scheme unsafe")
            ref[k] = r

    buckets, per_shard, ones = _layout(mask)
    nv = sum(w * n for w, n in buckets)
    kc = sum(n for _, n in buckets)
    nvp = (nv + 15) // 16 * 16

    # layout (r, d_in_block, bc, j): xt[b][r, p, bc, j] = xb[bc*128+j, r*128+p]
    x16 = x.astype(np.float16)
    xtb, x2tb = [], []
    for b in range(BSH):
        xb = x16[b * RB:(b + 1) * RB]
        xtb.append(np.ascontiguousarray(
            xb.reshape(NB, 128, 4, 128).transpose(2, 3, 0, 1)))
        x2tb.append(np.ascontiguousarray(
            np.clip((X2S * xb.astype(np.float64)) ** 2, 0, 240)
            .reshape(NB, 128, 4, 128).transpose(2, 3, 0, 1)
            .astype(ml_dtypes.float8_e4m3)))

    shard_w, metas = [], []
    for ks in range(KSH):
        # unused/promoted cols: every element PAD_LOGIT/D so the folded
        # constant sums to PAD_LOGIT (Sx2=1) -> exp ~ 0, never scattered
        w1c = np.full((nv, D), PAD_LOGIT / D, np.float64)
        w2c = np.zeros((nv, D), np.float64)
        col_cls = np.full(nv, -1, np.int64)
        col_mode = np.zeros(nv, np.int64)
        col_slot = np.zeros(nv, np.int64)
        kcls = np.full(kc, -1, np.int64)
        kref = np.zeros(kc, np.int64)
        off = koff = 0
        for w, ids in per_shard[ks]:
            for k in ids:
                if k >= 0:
                    c, r = counts[k], ref[k]
                    ms = [m for m in range(c) if m != r]
                    nm = len(ms)
                    w1c[off:off + nm] = (W1[k, ms] - W1[k, r]
                                         + (lc[k, ms] - lc[k, r])[:, None])
                    w2c[off:off + nm] = W2[k, ms] - W2[k, r]
                    # promoted classes: unused cols stay at exp->0
                    col_cls[off:off + nm] = k
                    col_mode[off:off + nm] = ms
                    kcls[koff] = k
                    kref[koff] = r
                col_slot[off:off + w] = koff
                off += w
                koff += 1
        # fp8 DoubleRow packing: (pair, p, i, nv) padded to nvp
        tmp = (w1c.T / (X2S * X2S)).reshape(2, 2, 128, nv) \
            .transpose(0, 2, 1, 3)
        w1pk = np.zeros((2, 128, 2, nvp), np.float64)
        w1pk[..., :nv] = tmp
        w1pk = np.clip(w1pk, -240, 240).reshape(2, 128, 2 * nvp) \
            .astype(ml_dtypes.float8_e4m3)
        shard_w.append({
            "w1": np.ascontiguousarray(w1pk),
            "w2": np.ascontiguousarray(
                w2c.T.astype(np.float16).reshape(4, 128, nv)),
        })
        metas.append((col_cls, col_mode, col_slot, kcls, kref))

    in_maps = []
    for cidx in range(NCORES):
        bs, ks = divmod(cidx, KSH)
        in_maps.append({"xt": xtb[bs], "x2t": x2tb[bs], **shard_w[ks]})
    return in_maps, buckets, metas, ones


def scatter_core(out, gamma, packed, rec, meta, rows):
    """Scatter one core's raw exps + per-class recips into out, applying
    gamma*rec on the host."""
    col_cls, col_mode, col_slot, kcls, kref = meta
    real = col_cls >= 0
    gpack = gamma[rows][:, col_cls[real]]
    out[rows, col_cls[real], col_mode[real]] = (
        packed[:, real] * rec[:, col_slot[real]] * gpack)
    realk = kcls >= 0
    out[rows, kcls[realk], kref[realk]] = (
        rec[:, realk] * gamma[rows][:, kcls[realk]])


_NC_CACHE = {}


def _get_nc(buckets):
    if buckets not in _NC_CACHE:
        _NC_CACHE[buckets] = build_bass(buckets)
    return _NC_CACHE[buckets]


def kernel(x, gamma_class, mu_pad, var_pad, pi_pad, mask, _trace=False):
    in_maps, buckets, metas, ones = prep_inputs(
        x, gamma_class, mu_pad, var_pad, pi_pad, mask)
    gamma_class = np.asarray(gamma_class, np.float32)
    out = np.zeros((B, K, M), np.float32)
    if len(ones):
        out[:, ones, 0] = gamma_class[:, ones]
    if not buckets:
        return out
    nc = _get_nc(buckets)
    res = bass_utils.run_bass_kernel_spmd(
        nc, in_maps, core_ids=list(range(NCORES)), trace=_trace)
    for cidx in range(NCORES):
        bs, ks = divmod(cidx, KSH)
        rows = slice(bs * RB, (bs + 1) * RB)
        scatter_core(out, gamma_class,
                     res.results[cidx]["out"].astype(np.float32),
                     res.results[cidx]["cf"].astype(np.float32),
                     metas[ks], rows)
    if _trace:
        kernel.last_results = res
    return out
